# revision 11
# baseline (speedup 1.0000x reference)
"""RGCN Aggregator (2-layer, basis-block-diagonal) on 8 TRN2 NeuronCores.

Algorithm (per layer):
    h = ent_embeds[node_id]                       (layer-0 input, via index composition)
    msg_e = blockdiag(W[edge_type_e]) @ h[src_e]  (64 blocks of 2x2)
    agg_n = (sum_{e: dst_e=n} msg_e) * norm_n + h_n @ loop_weight
    h'_n  = leakyrelu(agg_n, slope=(1/8+1/3)/2)

Distribution: nodes are assigned to 8 cores balanced by in-degree, so each
core owns ~E/8 edges keyed by dst ("shard edges across devices").  Each core
aggregates only its own nodes (no all-reduce); one chunked all-gather of h
between the two layers overlaps with layer-1 compute.

Per-edge math on device:
    msg = x * A[t] + swap2(x * Csw[t])    where swap2 swaps feature pairs
    A[t][2b+j]  = W[t][b][j][j],  Csw[t][2b] = W[t][b][0][1], Csw[t][2b+1] = W[t][b][1][0]
The swap2 commutes with the dst-sum, so it is applied once per 128-node bin
on the two PSUM accumulator halves instead of per edge.

x rows are fetched with [128,1]-offset indirect DMAs (the HW-validated form).
[A|Csw] per-edge rows are produced on the TensorEngine: a one-hot over the 256
(padded) relations is built with tensor_scalar(is_equal), PE-transposed, and
multiplied against the SBUF-resident relation table (AC_MODE="pe"); the
alternative AC_MODE="gather" fetches them with indirect DMAs like x.

Aggregation: per 128-edge chunk, a one-hot (edge x node-slot) matrix feeds
PSUM-accumulating PE matmuls over the bin's K chunks.

Host-side work is index bookkeeping only (bin assignment, edge permutation,
index composition node_id[src], weight-table relayout); all float math on the
edge/node data runs on device.
"""

import sys

for _p in ("/opt/trn_rl_repo",):
    if _p not in sys.path:
        sys.path.insert(0, _p)

import numpy as np

RRELU_SLOPE = (1.0 / 8.0 + 1.0 / 3.0) / 2.0
D = 128
P = 128
RPAD = 256  # relation table padded to 2 partition tiles

DEFAULT_HP = dict(
    N=100000,
    E=500000,
    R=230,
    NC=8,
    B=100,     # node bins per core (128 node slots each)
    AGCH=4,    # all-gather chunks (must divide B)
    G=5,       # bins per gather superblock (must divide B // AGCH)
    AC_BF16=True,   # relation table (and its matmuls / gathers) in bf16
    SL_BF16=True,   # self-loop matmul + transpose in bf16
    AC_MODE="pe",   # "pe" (one-hot matmuls) or "gather" (indirect DMA)
)


# ---------------------------------------------------------------- host prep
def _prepare(inputs, hp):
    """Index bookkeeping: node->(core,bin,slot), edge slot layout, tables."""
    N, E, R, NC, B = hp["N"], hp["E"], hp["R"], hp["NC"], hp["B"]
    AGCH = hp["AGCH"]
    BPC = B // AGCH

    node_id = np.asarray(inputs["node_id"]).astype(np.int64)
    edge_type = np.asarray(inputs["edge_type"]).astype(np.int64)
    src = np.asarray(inputs["src"]).astype(np.int64)
    dst = np.asarray(inputs["dst"]).astype(np.int64)
    norm = np.asarray(inputs["norm"], dtype=np.float32).reshape(N)
    weight = np.asarray(inputs["weight"], dtype=np.float32)
    loop_weight = np.asarray(inputs["loop_weight"], dtype=np.float32)
    ent = np.ascontiguousarray(np.asarray(inputs["ent_embeds"], dtype=np.float32))

    # --- node -> (core, bin, slot), balanced by in-degree --------------------
    deg = np.bincount(dst, minlength=N)
    order = np.argsort(-deg, kind="stable")
    rank = np.empty(N, np.int64)
    rank[order] = np.arange(N)
    core_of = (rank % NC).astype(np.int32)
    bin_of = ((rank // NC) % B).astype(np.int32)
    slot_of = (rank // (NC * B)).astype(np.int32)
    assert slot_of.max() < P, "too many node slots per bin"

    # row of node n in the all-gathered h table (chunk-major, then core)
    chunk_of = bin_of // BPC
    row_full = (
        chunk_of.astype(np.int64) * (NC * BPC * P)
        + core_of.astype(np.int64) * (BPC * P)
        + (bin_of % BPC).astype(np.int64) * P
        + slot_of
    ).astype(np.int32)

    # --- edges -> (core, bin, chunk k, partition p) --------------------------
    ecore = core_of[dst]
    ebin = bin_of[dst]
    key = ecore.astype(np.int64) * B + ebin
    perm_e = np.argsort(key, kind="stable")
    counts = np.bincount(key, minlength=NC * B)
    K = max(1, int(-(-counts.max() // P)))  # chunks of 128 edges per bin
    starts = np.zeros(NC * B, np.int64)
    starts[1:] = np.cumsum(counts)[:-1]
    j = np.arange(E, dtype=np.int64) - starts[key[perm_e]]
    p_ = (j % P).astype(np.int64)
    k_ = (j // P).astype(np.int64)
    c_ = ecore[perm_e].astype(np.int64)
    b_ = ebin[perm_e].astype(np.int64)

    src1 = np.zeros((NC, P, B, K), np.int32)
    src2 = np.zeros((NC, P, B, K), np.int32)
    etv = np.zeros((NC, P, B, K), np.int32)
    dstf = np.full((NC, P, B, K), -1.0, np.float32)
    src1[c_, p_, b_, k_] = node_id[src[perm_e]].astype(np.int32)
    src2[c_, p_, b_, k_] = row_full[src[perm_e]]
    etv[c_, p_, b_, k_] = edge_type[perm_e].astype(np.int32)
    dstf[c_, p_, b_, k_] = slot_of[dst[perm_e]].astype(np.float32)

    # --- per-node arrays (self-loop gather indices, norm), bin layout --------
    sl1 = np.zeros((NC, P, B), np.int32)
    nrm = np.zeros((NC, P, B), np.float32)
    sl1[core_of, slot_of, bin_of] = node_id.astype(np.int32)
    nrm[core_of, slot_of, bin_of] = norm

    # --- relation tables [RPAD, 256] = [A | Csw] per layer -------------------
    L = weight.shape[0]
    acsw = []
    for l in range(L):
        w = weight[l].reshape(R, D // 2, 2, 2)
        t = np.zeros((RPAD, 2 * D), np.float32)
        t[:R, 0:D:2] = w[:, :, 0, 0]
        t[:R, 1:D:2] = w[:, :, 1, 1]
        t[:R, D + 0 :: 2] = w[:, :, 0, 1]
        t[:R, D + 1 :: 2] = w[:, :, 1, 0]
        acsw.append(t)

    iotaf = np.tile(np.arange(P, dtype=np.float32), (P, K))     # [128, K*128]
    iotar = np.tile(np.arange(RPAD, dtype=np.float32), (P, 1))  # [128, 256]
    ident = np.eye(P, dtype=np.float32)

    meta = dict(core_of=core_of, bin_of=bin_of, slot_of=slot_of, K=K)

    if hp.get("AC_BF16"):
        import ml_dtypes

        ac_np = ml_dtypes.bfloat16
    else:
        ac_np = np.float32

    in_maps = []
    for c in range(NC):
        m = dict(
            ent=ent,
            acp0=acsw[0].astype(ac_np),
            acp1=(acsw[1] if L > 1 else acsw[0]).astype(ac_np),
            lw0=loop_weight[0],
            lw1=loop_weight[1] if L > 1 else loop_weight[0],
            srcidx0=np.ascontiguousarray(src1[c].reshape(P, B * K)),
            srcidx1=np.ascontiguousarray(src2[c].reshape(P, B * K)),
            etidx=np.ascontiguousarray(etv[c].reshape(P, B * K)),
            etf=np.ascontiguousarray(etv[c].reshape(P, B * K).astype(np.float32)),
            dstoff=np.ascontiguousarray(dstf[c].reshape(P, B * K)),
            slidx=np.ascontiguousarray(sl1[c]),
            normv=np.ascontiguousarray(nrm[c]),
            iotaf=iotaf,
            iotar=iotar,
            ident=ident,
            identb=ident.astype(ac_np),
        )
        in_maps.append(m)
    return in_maps, meta


# ---------------------------------------------------------------- device code
def build_program(hp, K):
    import concourse.bacc as bacc
    import concourse.mybir as mybir
    import concourse.tile as tile
    from concourse.bass import IndirectOffsetOnAxis

    f32 = mybir.dt.float32
    f32r = mybir.dt.float32r
    bf16 = mybir.dt.bfloat16
    i32 = mybir.dt.int32
    ac_dt = bf16 if hp.get("AC_BF16") else f32
    sl_dt = bf16 if hp.get("SL_BF16") else f32
    AluOp = mybir.AluOpType

    N, R, NC, B, AGCH, G = hp["N"], hp["R"], hp["NC"], hp["B"], hp["AGCH"], hp["G"]
    BPC = B // AGCH
    assert B % AGCH == 0 and BPC % G == 0
    NSB = B // G
    ROWS = B * P
    ac_pe = hp.get("AC_MODE", "pe") == "pe"

    nc = bacc.Bacc(
        "TRN2", target_bir_lowering=False, debug=False, num_devices=NC
    )

    ent = nc.declare_dram_parameter("ent", [N, D], f32, isOutput=False)
    acp = [
        nc.declare_dram_parameter("acp0", [RPAD, 2 * D], ac_dt, isOutput=False),
        nc.declare_dram_parameter("acp1", [RPAD, 2 * D], ac_dt, isOutput=False),
    ]
    lw = [
        nc.declare_dram_parameter("lw0", [D, D], f32, isOutput=False),
        nc.declare_dram_parameter("lw1", [D, D], f32, isOutput=False),
    ]
    srcidx = [
        nc.declare_dram_parameter("srcidx0", [P, B * K], i32, isOutput=False),
        nc.declare_dram_parameter("srcidx1", [P, B * K], i32, isOutput=False),
    ]
    etidx_d = nc.declare_dram_parameter("etidx", [P, B * K], i32, isOutput=False)
    etf_d = nc.declare_dram_parameter("etf", [P, B * K], f32, isOutput=False)
    dstoff_d = nc.declare_dram_parameter("dstoff", [P, B * K], f32, isOutput=False)
    slidx_d = nc.declare_dram_parameter("slidx", [P, B], i32, isOutput=False)
    normv_d = nc.declare_dram_parameter("normv", [P, B], f32, isOutput=False)
    iotaf_d = nc.declare_dram_parameter("iotaf", [P, K * P], f32, isOutput=False)
    iotar_d = nc.declare_dram_parameter("iotar", [P, RPAD], f32, isOutput=False)
    ident_d = nc.declare_dram_parameter("ident", [P, P], f32, isOutput=False)
    identb_d = nc.declare_dram_parameter("identb", [P, P], ac_dt, isOutput=False)
    out_d = nc.declare_dram_parameter("out", [ROWS, D], f32, isOutput=True)

    h1c = [nc.dram_tensor(f"h1c{i}", [BPC * P, D], f32) for i in range(AGCH)]
    h1f = nc.dram_tensor("h1f", [AGCH * NC * BPC * P, D], f32, addr_space="Shared")

    with tile.TileContext(nc) as tc:
        with (
            tc.tile_pool(name="const", bufs=1) as cpool,
            tc.tile_pool(name="gath", bufs=2) as gpool,
            tc.tile_pool(name="work", bufs=3) as wpool,
            tc.tile_pool(name="epi", bufs=3) as epool,
            tc.tile_pool(name="psA", bufs=2, space="PSUM") as psA_pool,
            tc.tile_pool(name="psB", bufs=1, space="PSUM") as psB_pool,
            tc.tile_pool(name="psT", bufs=2, space="PSUM") as psT_pool,
            tc.tile_pool(name="psC", bufs=2, space="PSUM") as psC_pool,
        ):
            def load_const(shape, dt_, dram, tag):
                t = cpool.tile(shape, dt_, tag=tag)
                nc.sync.dma_start(out=t[:], in_=dram[:])
                return t

            iota_sb = load_const([P, K * P], f32, iotaf_d, "iota")
            ident_sb = load_const([P, P], f32, ident_d, "ident")
            identb_sb = load_const([P, P], ac_dt, identb_d, "identb")
            iotar_sb = load_const([P, RPAD], f32, iotar_d, "iotar")
            lw_sb = [
                load_const([P, D], f32, lw[0], "lw0"),
                load_const([P, D], f32, lw[1], "lw1"),
            ]
            lwc = []
            for l in range(2):
                t = cpool.tile([P, D], sl_dt if sl_dt != f32 else f32r,
                               tag=f"lwb{l}")
                nc.vector.tensor_copy(out=t[:], in_=lw_sb[l][:])
                lwc.append(t)
            lw_sb = lwc
            # relation tables resident in SBUF (two 128-partition halves)
            T_sb = []
            for l in range(2):
                t0 = load_const([P, 2 * D], ac_dt, acp[l][0:P, :], f"T{l}0")
                t1 = load_const([P, 2 * D], ac_dt, acp[l][P : 2 * P, :], f"T{l}1")
                T_sb.append((t0, t1))

            et_sb = load_const([P, B * K], i32, etidx_d, "et")
            etf_sb = load_const([P, B * K], f32, etf_d, "etf")
            dst_sb = load_const([P, B * K], f32, dstoff_d, "dst")
            sl_sb = load_const([P, B], i32, slidx_d, "sl")
            nrm_sb = load_const([P, B], f32, normv_d, "nrm")
            src_sb = [
                load_const([P, B * K], i32, srcidx[0], "src0"),
                load_const([P, B * K], i32, srcidx[1], "src1"),
            ]

            def layer(l, table, out_chunks):
                for sb in range(NSB):
                    b0 = sb * G
                    ci = b0 // BPC
                    # ---- x gathers: [P,1] indirect calls ----
                    xq = gpool.tile([P, G * K, D], f32, tag="xq")
                    for j in range(G * K):
                        nc.gpsimd.indirect_dma_start(
                            out=xq[:, j, :],
                            out_offset=None,
                            in_=table[:],
                            in_offset=IndirectOffsetOnAxis(
                                ap=src_sb[l][:, b0 * K + j : b0 * K + j + 1], axis=0
                            ),
                        )
                    if not ac_pe:
                        acq = gpool.tile([P, G * K, 2 * D], ac_dt, tag="acq")
                        for j in range(G * K):
                            nc.gpsimd.indirect_dma_start(
                                out=acq[:, j, :],
                                out_offset=None,
                                in_=acp[l][:],
                                in_offset=IndirectOffsetOnAxis(
                                    ap=et_sb[:, b0 * K + j : b0 * K + j + 1], axis=0
                                ),
                            )
                    # ---- self-loop input rows for G bins ----
                    h0g = gpool.tile([P, G, D], sl_dt, tag="h0g")
                    if l == 0:
                        h0gf = gpool.tile([P, G, D], f32, tag="h0gf")
                        for g in range(G):
                            nc.gpsimd.indirect_dma_start(
                                out=h0gf[:, g, :],
                                out_offset=None,
                                in_=ent[:],
                                in_offset=IndirectOffsetOnAxis(
                                    ap=sl_sb[:, b0 + g : b0 + g + 1], axis=0
                                ),
                            )
                        nc.any.tensor_copy(out=h0g[:], in_=h0gf[:])
                    else:
                        src_rows = h1c[ci][(b0 % BPC) * P : (b0 % BPC + G) * P, :]
                        if sl_dt != f32:
                            h0gf = gpool.tile([P, G, D], f32, tag="h0gf")
                            nc.sync.dma_start(
                                out=h0gf[:],
                                in_=src_rows.rearrange("(g p) d -> p g d", p=P),
                            )
                            nc.any.tensor_copy(out=h0g[:], in_=h0gf[:])
                        else:
                            nc.sync.dma_start(
                                out=h0g[:],
                                in_=src_rows.rearrange("(g p) d -> p g d", p=P),
                            )

                    for g in range(G):
                        b = b0 + g
                        msg = wpool.tile([P, K, 2 * D], f32r, tag="msg")
                        oh = wpool.tile([P, K, P], f32r, tag="oh")
                        psA = psA_pool.tile([P, 2 * D], f32, tag="psA")
                        for k in range(K):
                            col = b * K + k
                            if ac_pe:
                                # one-hot over relations -> PE "gather" of [A|Csw]
                                ohR = wpool.tile([P, RPAD], ac_dt, tag="ohR")
                                nc.any.tensor_scalar(
                                    out=ohR[:],
                                    in0=iotar_sb[:],
                                    scalar1=etf_sb[:, col : col + 1],
                                    scalar2=None,
                                    op0=AluOp.is_equal,
                                )
                                psRT = psT_pool.tile([P, RPAD], ac_dt, tag="psRT")
                                nc.tensor.transpose(
                                    out=psRT[:, 0:P], in_=ohR[:, 0:P],
                                    identity=identb_sb[:],
                                )
                                nc.tensor.transpose(
                                    out=psRT[:, P:RPAD], in_=ohR[:, P:RPAD],
                                    identity=identb_sb[:],
                                )
                                ohT = wpool.tile([P, RPAD], ac_dt, tag="ohT")
                                nc.any.tensor_copy(out=ohT[:], in_=psRT[:])
                                psAC = psC_pool.tile([P, 2 * D], f32, tag="psAC")
                                nc.tensor.matmul(
                                    out=psAC[:], lhsT=ohT[:, 0:P], rhs=T_sb[l][0][:],
                                    start=True, stop=False,
                                )
                                nc.tensor.matmul(
                                    out=psAC[:], lhsT=ohT[:, P:RPAD], rhs=T_sb[l][1][:],
                                    start=False, stop=True,
                                )
                                ac_lo = psAC[:, 0:D]
                                ac_hi = psAC[:, D : 2 * D]
                            else:
                                ac_lo = acq[:, g * K + k, 0:D]
                                ac_hi = acq[:, g * K + k, D : 2 * D]
                            nc.vector.tensor_tensor(
                                out=msg[:, k, 0:D],
                                in0=xq[:, g * K + k, :],
                                in1=ac_lo,
                                op=AluOp.mult,
                            )
                            nc.vector.tensor_tensor(
                                out=msg[:, k, D : 2 * D],
                                in0=xq[:, g * K + k, :],
                                in1=ac_hi,
                                op=AluOp.mult,
                            )
                            nc.any.tensor_scalar(
                                out=oh[:, k, :],
                                in0=iota_sb[:, k * P : (k + 1) * P],
                                scalar1=dst_sb[:, col : col + 1],
                                scalar2=None,
                                op0=AluOp.is_equal,
                            )
                            nc.tensor.matmul(
                                out=psA[:],
                                lhsT=oh[:, k, :],
                                rhs=msg[:, k, :],
                                start=(k == 0),
                                stop=(k == K - 1),
                            )
                        # ---- self-loop term ----
                        psT = psB_pool.tile([P, P], sl_dt, tag="psT")
                        nc.tensor.transpose(
                            out=psT[:], in_=h0g[:, g, :],
                            identity=(identb_sb[:] if sl_dt == ac_dt else ident_sb[:]),
                        )
                        h0T = epool.tile([P, P], sl_dt if sl_dt != f32 else f32r,
                                         tag="h0T")
                        nc.scalar.copy(out=h0T[:], in_=psT[:])
                        psL = psB_pool.tile([P, P], f32, tag="psL")
                        nc.tensor.matmul(
                            out=psL[:], lhsT=h0T[:], rhs=lw_sb[l][:],
                            start=True, stop=True,
                        )
                        # ---- epilogue ----
                        s = epool.tile([P, 2 * D], f32, tag="s")
                        nc.scalar.copy(out=s[:], in_=psA[:])
                        t = epool.tile([P, D], f32, tag="t")
                        nc.vector.tensor_tensor(
                            out=t[:, 0:D:2], in0=s[:, 0:D:2],
                            in1=s[:, D + 1 : 2 * D : 2], op=AluOp.add,
                        )
                        nc.vector.tensor_tensor(
                            out=t[:, 1:D:2], in0=s[:, 1:D:2],
                            in1=s[:, D : 2 * D : 2], op=AluOp.add,
                        )
                        t3 = epool.tile([P, D], f32, tag="t3")
                        nc.vector.scalar_tensor_tensor(
                            out=t3[:], in0=t[:], scalar=nrm_sb[:, b : b + 1],
                            in1=psL[:], op0=AluOp.mult, op1=AluOp.add,
                        )
                        ho = epool.tile([P, D], f32, tag="ho")
                        nc.vector.scalar_tensor_tensor(
                            out=ho[:], in0=t3[:], scalar=float(RRELU_SLOPE),
                            in1=t3[:], op0=AluOp.mult, op1=AluOp.max,
                        )
                        nc.sync.dma_start(
                            out=out_chunks[b // BPC][
                                (b % BPC) * P : (b % BPC + 1) * P, :
                            ],
                            in_=ho[:],
                        )

            layer(0, ent, h1c)
            for i in range(AGCH):
                if hp.get("MOCK_COLLECTIVE"):
                    for c in range(NC):
                        nc.sync.dma_start(
                            out=h1f[
                                (i * NC + c) * BPC * P : (i * NC + c + 1) * BPC * P, :
                            ],
                            in_=h1c[i][:],
                        )
                else:
                    nc.gpsimd.collective_compute(
                        "AllGather",
                        mybir.AluOpType.bypass,
                        replica_groups=[list(range(NC))],
                        ins=[h1c[i][:]],
                        outs=[h1f[i * NC * BPC * P : (i + 1) * NC * BPC * P, :]],
                    )
            out_chunks = [
                out_d[i * BPC * P : (i + 1) * BPC * P, :] for i in range(AGCH)
            ]
            layer(1, h1f, out_chunks)

    nc.finalize()
    return nc


# ---------------------------------------------------------------- entrypoint
_PROGRAM_CACHE: dict = {}


def _get_program(hp, K):
    key = (K, hp.get("AC_BF16"), hp.get("SL_BF16"), hp.get("AC_MODE"),
           hp.get("MOCK_COLLECTIVE"), hp["B"], hp["G"], hp["AGCH"], hp["NC"])
    if key not in _PROGRAM_CACHE:
        _PROGRAM_CACHE[key] = build_program(hp, K)
    return _PROGRAM_CACHE[key]


def _run(inputs, hp, trace=False):
    from concourse.bass_utils import run_bass_kernel_spmd

    in_maps, meta = _prepare(inputs, hp)
    nc = _get_program(hp, meta["K"])
    res = run_bass_kernel_spmd(
        nc, in_maps, core_ids=list(range(hp["NC"])), trace=trace
    )
    allout = np.stack([r["out"] for r in res.results])
    core_of, bin_of, slot_of = meta["core_of"], meta["bin_of"], meta["slot_of"]
    out = allout[core_of, bin_of * P + slot_of].astype(np.float32)
    return out, res


def kernel(**inputs) -> np.ndarray:
    out, _ = _run(inputs, DEFAULT_HP)
    return out


# revision 12
# speedup vs baseline: 1.0739x; 1.0739x over previous
"""RGCN Aggregator (2-layer, basis-block-diagonal) on 8 TRN2 NeuronCores.

Algorithm (per layer):
    h = ent_embeds[node_id]                       (layer-0 input, via index composition)
    msg_e = blockdiag(W[edge_type_e]) @ h[src_e]  (64 blocks of 2x2)
    agg_n = (sum_{e: dst_e=n} msg_e) * norm_n + h_n @ loop_weight
    h'_n  = leakyrelu(agg_n, slope=(1/8+1/3)/2)

Distribution: nodes are assigned to 8 cores balanced by in-degree, so each
core owns ~E/8 edges keyed by dst ("shard edges across devices").  Each core
aggregates only its own nodes (no all-reduce); one chunked all-gather of h
between the two layers overlaps with layer-1 compute.

Per-edge math on device:
    msg = x * A[t] + swap2(x * Csw[t])    where swap2 swaps feature pairs
    A[t][2b+j]  = W[t][b][j][j],  Csw[t][2b] = W[t][b][0][1], Csw[t][2b+1] = W[t][b][1][0]
The swap2 commutes with the dst-sum, so it is applied once per 128-node bin
on the two PSUM accumulator halves instead of per edge.

x rows are fetched with [128,1]-offset indirect DMAs (the HW-validated form).
[A|Csw] per-edge rows are produced on the TensorEngine: a one-hot over the 256
(padded) relations is built with tensor_scalar(is_equal), PE-transposed, and
multiplied against the SBUF-resident relation table (AC_MODE="pe"); the
alternative AC_MODE="gather" fetches them with indirect DMAs like x.

Aggregation: per 128-edge chunk, a one-hot (edge x node-slot) matrix feeds
PSUM-accumulating PE matmuls over the bin's K chunks.

Host-side work is index bookkeeping only (bin assignment, edge permutation,
index composition node_id[src], weight-table relayout); all float math on the
edge/node data runs on device.
"""

import sys

for _p in ("/opt/trn_rl_repo",):
    if _p not in sys.path:
        sys.path.insert(0, _p)

import numpy as np

RRELU_SLOPE = (1.0 / 8.0 + 1.0 / 3.0) / 2.0
D = 128
P = 128
RPAD = 256  # relation table padded to 2 partition tiles

DEFAULT_HP = dict(
    N=100000,
    E=500000,
    R=230,
    NC=8,
    B=100,     # node bins per core (128 node slots each)
    AGCH=10,   # all-gather chunks (must divide B)
    G=5,       # bins per gather superblock (must divide B // AGCH)
    AC_BF16=True,   # relation table (and its matmuls / gathers) in bf16
    SL_BF16=True,   # self-loop matmul + transpose in bf16
    AC_MODE="pe",   # "pe" (one-hot matmuls) or "gather" (indirect DMA)
)


# ---------------------------------------------------------------- host prep
def _prepare(inputs, hp):
    """Index bookkeeping: node->(core,bin,slot), edge slot layout, tables."""
    N, E, R, NC, B = hp["N"], hp["E"], hp["R"], hp["NC"], hp["B"]
    AGCH = hp["AGCH"]
    BPC = B // AGCH

    node_id = np.asarray(inputs["node_id"]).astype(np.int64)
    edge_type = np.asarray(inputs["edge_type"]).astype(np.int64)
    src = np.asarray(inputs["src"]).astype(np.int64)
    dst = np.asarray(inputs["dst"]).astype(np.int64)
    norm = np.asarray(inputs["norm"], dtype=np.float32).reshape(N)
    weight = np.asarray(inputs["weight"], dtype=np.float32)
    loop_weight = np.asarray(inputs["loop_weight"], dtype=np.float32)
    ent = np.ascontiguousarray(np.asarray(inputs["ent_embeds"], dtype=np.float32))

    # --- node -> (core, bin, slot), balanced by in-degree --------------------
    deg = np.bincount(dst, minlength=N)
    order = np.argsort(-deg, kind="stable")
    rank = np.empty(N, np.int64)
    rank[order] = np.arange(N)
    core_of = (rank % NC).astype(np.int32)
    bin_of = ((rank // NC) % B).astype(np.int32)
    slot_of = (rank // (NC * B)).astype(np.int32)
    assert slot_of.max() < P, "too many node slots per bin"

    # row of node n in the all-gathered h table (chunk-major, then core)
    chunk_of = bin_of // BPC
    row_full = (
        chunk_of.astype(np.int64) * (NC * BPC * P)
        + core_of.astype(np.int64) * (BPC * P)
        + (bin_of % BPC).astype(np.int64) * P
        + slot_of
    ).astype(np.int32)

    # --- edges -> (core, bin, chunk k, partition p) --------------------------
    ecore = core_of[dst]
    ebin = bin_of[dst]
    key = ecore.astype(np.int64) * B + ebin
    perm_e = np.argsort(key, kind="stable")
    counts = np.bincount(key, minlength=NC * B)
    K = max(1, int(-(-counts.max() // P)))  # chunks of 128 edges per bin
    starts = np.zeros(NC * B, np.int64)
    starts[1:] = np.cumsum(counts)[:-1]
    j = np.arange(E, dtype=np.int64) - starts[key[perm_e]]
    p_ = (j % P).astype(np.int64)
    k_ = (j // P).astype(np.int64)
    c_ = ecore[perm_e].astype(np.int64)
    b_ = ebin[perm_e].astype(np.int64)

    src1 = np.zeros((NC, P, B, K), np.int32)
    src2 = np.zeros((NC, P, B, K), np.int32)
    etv = np.zeros((NC, P, B, K), np.int32)
    dstf = np.full((NC, P, B, K), -1.0, np.float32)
    src1[c_, p_, b_, k_] = node_id[src[perm_e]].astype(np.int32)
    src2[c_, p_, b_, k_] = row_full[src[perm_e]]
    etv[c_, p_, b_, k_] = edge_type[perm_e].astype(np.int32)
    dstf[c_, p_, b_, k_] = slot_of[dst[perm_e]].astype(np.float32)

    # --- per-node arrays (self-loop gather indices, norm), bin layout --------
    sl1 = np.zeros((NC, P, B), np.int32)
    nrm = np.zeros((NC, P, B), np.float32)
    sl1[core_of, slot_of, bin_of] = node_id.astype(np.int32)
    nrm[core_of, slot_of, bin_of] = norm

    # --- relation tables [RPAD, 256] = [A | Csw] per layer -------------------
    L = weight.shape[0]
    acsw = []
    for l in range(L):
        w = weight[l].reshape(R, D // 2, 2, 2)
        t = np.zeros((RPAD, 2 * D), np.float32)
        t[:R, 0:D:2] = w[:, :, 0, 0]
        t[:R, 1:D:2] = w[:, :, 1, 1]
        t[:R, D + 0 :: 2] = w[:, :, 0, 1]
        t[:R, D + 1 :: 2] = w[:, :, 1, 0]
        acsw.append(t)

    iotaf = np.tile(np.arange(P, dtype=np.float32), (P, K))     # [128, K*128]
    iotar = np.tile(np.arange(RPAD, dtype=np.float32), (P, 1))  # [128, 256]
    ident = np.eye(P, dtype=np.float32)

    meta = dict(core_of=core_of, bin_of=bin_of, slot_of=slot_of, K=K)

    if hp.get("AC_BF16"):
        import ml_dtypes

        ac_np = ml_dtypes.bfloat16
    else:
        ac_np = np.float32

    in_maps = []
    for c in range(NC):
        m = dict(
            ent=ent,
            acp0=acsw[0].astype(ac_np),
            acp1=(acsw[1] if L > 1 else acsw[0]).astype(ac_np),
            lw0=loop_weight[0],
            lw1=loop_weight[1] if L > 1 else loop_weight[0],
            srcidx0=np.ascontiguousarray(src1[c].reshape(P, B * K)),
            srcidx1=np.ascontiguousarray(src2[c].reshape(P, B * K)),
            etidx=np.ascontiguousarray(etv[c].reshape(P, B * K)),
            etf=np.ascontiguousarray(etv[c].reshape(P, B * K).astype(np.float32)),
            dstoff=np.ascontiguousarray(dstf[c].reshape(P, B * K)),
            slidx=np.ascontiguousarray(sl1[c]),
            normv=np.ascontiguousarray(nrm[c]),
            iotaf=iotaf,
            iotar=iotar,
            ident=ident,
            identb=ident.astype(ac_np),
        )
        in_maps.append(m)
    return in_maps, meta


# ---------------------------------------------------------------- device code
def build_program(hp, K):
    import concourse.bacc as bacc
    import concourse.mybir as mybir
    import concourse.tile as tile
    from concourse.bass import IndirectOffsetOnAxis

    f32 = mybir.dt.float32
    f32r = mybir.dt.float32r
    bf16 = mybir.dt.bfloat16
    i32 = mybir.dt.int32
    ac_dt = bf16 if hp.get("AC_BF16") else f32
    sl_dt = bf16 if hp.get("SL_BF16") else f32
    AluOp = mybir.AluOpType

    N, R, NC, B, AGCH, G = hp["N"], hp["R"], hp["NC"], hp["B"], hp["AGCH"], hp["G"]
    BPC = B // AGCH
    assert B % AGCH == 0 and BPC % G == 0
    NSB = B // G
    ROWS = B * P
    ac_pe = hp.get("AC_MODE", "pe") == "pe"

    nc = bacc.Bacc(
        "TRN2", target_bir_lowering=False, debug=False, num_devices=NC
    )

    ent = nc.declare_dram_parameter("ent", [N, D], f32, isOutput=False)
    acp = [
        nc.declare_dram_parameter("acp0", [RPAD, 2 * D], ac_dt, isOutput=False),
        nc.declare_dram_parameter("acp1", [RPAD, 2 * D], ac_dt, isOutput=False),
    ]
    lw = [
        nc.declare_dram_parameter("lw0", [D, D], f32, isOutput=False),
        nc.declare_dram_parameter("lw1", [D, D], f32, isOutput=False),
    ]
    srcidx = [
        nc.declare_dram_parameter("srcidx0", [P, B * K], i32, isOutput=False),
        nc.declare_dram_parameter("srcidx1", [P, B * K], i32, isOutput=False),
    ]
    etidx_d = nc.declare_dram_parameter("etidx", [P, B * K], i32, isOutput=False)
    etf_d = nc.declare_dram_parameter("etf", [P, B * K], f32, isOutput=False)
    dstoff_d = nc.declare_dram_parameter("dstoff", [P, B * K], f32, isOutput=False)
    slidx_d = nc.declare_dram_parameter("slidx", [P, B], i32, isOutput=False)
    normv_d = nc.declare_dram_parameter("normv", [P, B], f32, isOutput=False)
    iotaf_d = nc.declare_dram_parameter("iotaf", [P, K * P], f32, isOutput=False)
    iotar_d = nc.declare_dram_parameter("iotar", [P, RPAD], f32, isOutput=False)
    ident_d = nc.declare_dram_parameter("ident", [P, P], f32, isOutput=False)
    identb_d = nc.declare_dram_parameter("identb", [P, P], ac_dt, isOutput=False)
    out_d = nc.declare_dram_parameter("out", [ROWS, D], f32, isOutput=True)

    h1c = [nc.dram_tensor(f"h1c{i}", [BPC * P, D], f32) for i in range(AGCH)]
    h1f = nc.dram_tensor("h1f", [AGCH * NC * BPC * P, D], f32, addr_space="Shared")

    with tile.TileContext(nc) as tc:
        with (
            tc.tile_pool(name="const", bufs=1) as cpool,
            tc.tile_pool(name="gath", bufs=2) as gpool,
            tc.tile_pool(name="work", bufs=3) as wpool,
            tc.tile_pool(name="epi", bufs=3) as epool,
            tc.tile_pool(name="psA", bufs=2, space="PSUM") as psA_pool,
            tc.tile_pool(name="psB", bufs=1, space="PSUM") as psB_pool,
            tc.tile_pool(name="psT", bufs=2, space="PSUM") as psT_pool,
            tc.tile_pool(name="psC", bufs=2, space="PSUM") as psC_pool,
        ):
            def load_const(shape, dt_, dram, tag):
                t = cpool.tile(shape, dt_, tag=tag)
                nc.sync.dma_start(out=t[:], in_=dram[:])
                return t

            iota_sb = load_const([P, K * P], f32, iotaf_d, "iota")
            ident_sb = load_const([P, P], f32, ident_d, "ident")
            identb_sb = load_const([P, P], ac_dt, identb_d, "identb")
            iotar_sb = load_const([P, RPAD], f32, iotar_d, "iotar")
            lw_sb = [
                load_const([P, D], f32, lw[0], "lw0"),
                load_const([P, D], f32, lw[1], "lw1"),
            ]
            lwc = []
            for l in range(2):
                t = cpool.tile([P, D], sl_dt if sl_dt != f32 else f32r,
                               tag=f"lwb{l}")
                nc.vector.tensor_copy(out=t[:], in_=lw_sb[l][:])
                lwc.append(t)
            lw_sb = lwc
            # relation tables resident in SBUF (two 128-partition halves)
            T_sb = []
            for l in range(2):
                t0 = load_const([P, 2 * D], ac_dt, acp[l][0:P, :], f"T{l}0")
                t1 = load_const([P, 2 * D], ac_dt, acp[l][P : 2 * P, :], f"T{l}1")
                T_sb.append((t0, t1))

            et_sb = load_const([P, B * K], i32, etidx_d, "et")
            etf_sb = load_const([P, B * K], f32, etf_d, "etf")
            dst_sb = load_const([P, B * K], f32, dstoff_d, "dst")
            sl_sb = load_const([P, B], i32, slidx_d, "sl")
            nrm_sb = load_const([P, B], f32, normv_d, "nrm")
            src_sb = [
                load_const([P, B * K], i32, srcidx[0], "src0"),
                load_const([P, B * K], i32, srcidx[1], "src1"),
            ]

            def layer(l, table, out_chunks):
                for sb in range(NSB):
                    b0 = sb * G
                    ci = b0 // BPC
                    # ---- x gathers: [P,1] indirect calls ----
                    xq = gpool.tile([P, G * K, D], f32, tag="xq")
                    for j in range(G * K):
                        nc.gpsimd.indirect_dma_start(
                            out=xq[:, j, :],
                            out_offset=None,
                            in_=table[:],
                            in_offset=IndirectOffsetOnAxis(
                                ap=src_sb[l][:, b0 * K + j : b0 * K + j + 1], axis=0
                            ),
                        )
                    if not ac_pe:
                        acq = gpool.tile([P, G * K, 2 * D], ac_dt, tag="acq")
                        for j in range(G * K):
                            nc.gpsimd.indirect_dma_start(
                                out=acq[:, j, :],
                                out_offset=None,
                                in_=acp[l][:],
                                in_offset=IndirectOffsetOnAxis(
                                    ap=et_sb[:, b0 * K + j : b0 * K + j + 1], axis=0
                                ),
                            )
                    # ---- self-loop input rows for G bins ----
                    h0g = gpool.tile([P, G, D], sl_dt, tag="h0g")
                    if l == 0:
                        h0gf = gpool.tile([P, G, D], f32, tag="h0gf")
                        for g in range(G):
                            nc.gpsimd.indirect_dma_start(
                                out=h0gf[:, g, :],
                                out_offset=None,
                                in_=ent[:],
                                in_offset=IndirectOffsetOnAxis(
                                    ap=sl_sb[:, b0 + g : b0 + g + 1], axis=0
                                ),
                            )
                        nc.any.tensor_copy(out=h0g[:], in_=h0gf[:])
                    else:
                        src_rows = h1c[ci][(b0 % BPC) * P : (b0 % BPC + G) * P, :]
                        if sl_dt != f32:
                            h0gf = gpool.tile([P, G, D], f32, tag="h0gf")
                            nc.sync.dma_start(
                                out=h0gf[:],
                                in_=src_rows.rearrange("(g p) d -> p g d", p=P),
                            )
                            nc.any.tensor_copy(out=h0g[:], in_=h0gf[:])
                        else:
                            nc.sync.dma_start(
                                out=h0g[:],
                                in_=src_rows.rearrange("(g p) d -> p g d", p=P),
                            )

                    for g in range(G):
                        b = b0 + g
                        msg = wpool.tile([P, K, 2 * D], f32r, tag="msg")
                        oh = wpool.tile([P, K, P], f32r, tag="oh")
                        psA = psA_pool.tile([P, 2 * D], f32, tag="psA")
                        for k in range(K):
                            col = b * K + k
                            if ac_pe:
                                # one-hot over relations -> PE "gather" of [A|Csw]
                                ohR = wpool.tile([P, RPAD], ac_dt, tag="ohR")
                                nc.any.tensor_scalar(
                                    out=ohR[:],
                                    in0=iotar_sb[:],
                                    scalar1=etf_sb[:, col : col + 1],
                                    scalar2=None,
                                    op0=AluOp.is_equal,
                                )
                                psRT = psT_pool.tile([P, RPAD], ac_dt, tag="psRT")
                                nc.tensor.transpose(
                                    out=psRT[:, 0:P], in_=ohR[:, 0:P],
                                    identity=identb_sb[:],
                                )
                                nc.tensor.transpose(
                                    out=psRT[:, P:RPAD], in_=ohR[:, P:RPAD],
                                    identity=identb_sb[:],
                                )
                                ohT = wpool.tile([P, RPAD], ac_dt, tag="ohT")
                                nc.any.tensor_copy(out=ohT[:], in_=psRT[:])
                                psAC = psC_pool.tile([P, 2 * D], f32, tag="psAC")
                                nc.tensor.matmul(
                                    out=psAC[:], lhsT=ohT[:, 0:P], rhs=T_sb[l][0][:],
                                    start=True, stop=False,
                                )
                                nc.tensor.matmul(
                                    out=psAC[:], lhsT=ohT[:, P:RPAD], rhs=T_sb[l][1][:],
                                    start=False, stop=True,
                                )
                                ac_lo = psAC[:, 0:D]
                                ac_hi = psAC[:, D : 2 * D]
                            else:
                                ac_lo = acq[:, g * K + k, 0:D]
                                ac_hi = acq[:, g * K + k, D : 2 * D]
                            nc.vector.tensor_tensor(
                                out=msg[:, k, 0:D],
                                in0=xq[:, g * K + k, :],
                                in1=ac_lo,
                                op=AluOp.mult,
                            )
                            nc.vector.tensor_tensor(
                                out=msg[:, k, D : 2 * D],
                                in0=xq[:, g * K + k, :],
                                in1=ac_hi,
                                op=AluOp.mult,
                            )
                            nc.any.tensor_scalar(
                                out=oh[:, k, :],
                                in0=iota_sb[:, k * P : (k + 1) * P],
                                scalar1=dst_sb[:, col : col + 1],
                                scalar2=None,
                                op0=AluOp.is_equal,
                            )
                            nc.tensor.matmul(
                                out=psA[:],
                                lhsT=oh[:, k, :],
                                rhs=msg[:, k, :],
                                start=(k == 0),
                                stop=(k == K - 1),
                            )
                        # ---- self-loop term ----
                        psT = psB_pool.tile([P, P], sl_dt, tag="psT")
                        nc.tensor.transpose(
                            out=psT[:], in_=h0g[:, g, :],
                            identity=(identb_sb[:] if sl_dt == ac_dt else ident_sb[:]),
                        )
                        h0T = epool.tile([P, P], sl_dt if sl_dt != f32 else f32r,
                                         tag="h0T")
                        nc.scalar.copy(out=h0T[:], in_=psT[:])
                        psL = psB_pool.tile([P, P], f32, tag="psL")
                        nc.tensor.matmul(
                            out=psL[:], lhsT=h0T[:], rhs=lw_sb[l][:],
                            start=True, stop=True,
                        )
                        # ---- epilogue ----
                        s = epool.tile([P, 2 * D], f32, tag="s")
                        nc.scalar.copy(out=s[:], in_=psA[:])
                        t = epool.tile([P, D], f32, tag="t")
                        nc.vector.tensor_tensor(
                            out=t[:, 0:D:2], in0=s[:, 0:D:2],
                            in1=s[:, D + 1 : 2 * D : 2], op=AluOp.add,
                        )
                        nc.vector.tensor_tensor(
                            out=t[:, 1:D:2], in0=s[:, 1:D:2],
                            in1=s[:, D : 2 * D : 2], op=AluOp.add,
                        )
                        t3 = epool.tile([P, D], f32, tag="t3")
                        nc.vector.scalar_tensor_tensor(
                            out=t3[:], in0=t[:], scalar=nrm_sb[:, b : b + 1],
                            in1=psL[:], op0=AluOp.mult, op1=AluOp.add,
                        )
                        ho = epool.tile([P, D], f32, tag="ho")
                        nc.vector.scalar_tensor_tensor(
                            out=ho[:], in0=t3[:], scalar=float(RRELU_SLOPE),
                            in1=t3[:], op0=AluOp.mult, op1=AluOp.max,
                        )
                        nc.sync.dma_start(
                            out=out_chunks[b // BPC][
                                (b % BPC) * P : (b % BPC + 1) * P, :
                            ],
                            in_=ho[:],
                        )

            layer(0, ent, h1c)
            for i in range(AGCH):
                if hp.get("MOCK_COLLECTIVE"):
                    for c in range(NC):
                        nc.sync.dma_start(
                            out=h1f[
                                (i * NC + c) * BPC * P : (i * NC + c + 1) * BPC * P, :
                            ],
                            in_=h1c[i][:],
                        )
                else:
                    nc.gpsimd.collective_compute(
                        "AllGather",
                        mybir.AluOpType.bypass,
                        replica_groups=[list(range(NC))],
                        ins=[h1c[i][:]],
                        outs=[h1f[i * NC * BPC * P : (i + 1) * NC * BPC * P, :]],
                    )
            out_chunks = [
                out_d[i * BPC * P : (i + 1) * BPC * P, :] for i in range(AGCH)
            ]
            layer(1, h1f, out_chunks)

    nc.finalize()
    return nc


# ---------------------------------------------------------------- entrypoint
_PROGRAM_CACHE: dict = {}


def _get_program(hp, K):
    key = (K, hp.get("AC_BF16"), hp.get("SL_BF16"), hp.get("AC_MODE"),
           hp.get("MOCK_COLLECTIVE"), hp["B"], hp["G"], hp["AGCH"], hp["NC"])
    if key not in _PROGRAM_CACHE:
        _PROGRAM_CACHE[key] = build_program(hp, K)
    return _PROGRAM_CACHE[key]


def _run(inputs, hp, trace=False):
    from concourse.bass_utils import run_bass_kernel_spmd

    in_maps, meta = _prepare(inputs, hp)
    nc = _get_program(hp, meta["K"])
    res = run_bass_kernel_spmd(
        nc, in_maps, core_ids=list(range(hp["NC"])), trace=trace
    )
    allout = np.stack([r["out"] for r in res.results])
    core_of, bin_of, slot_of = meta["core_of"], meta["bin_of"], meta["slot_of"]
    out = allout[core_of, bin_of * P + slot_of].astype(np.float32)
    return out, res


def kernel(**inputs) -> np.ndarray:
    out, _ = _run(inputs, DEFAULT_HP)
    return out
